# revision 30
# baseline (speedup 1.0000x reference)
"""Trainium2 Bass kernel for nn_AttentionHead_17042430231165.

out = softmax(min((x@wq.T+qb)@(x@wk.T+kb).T / 256, tri)) @ (x@wv.T+vb)
  x [32, 8192], wk/wq [256, 8192], wv [8192, 8192], tri [32, 32]

Sharding (8 cores):
  - wv rows (VAL) sharded: 1024 rows/core -> each core computes out[:, c*1024:(c+1)*1024]
  - wk/wq fully REPLICATED on every core; scores and softmax computed
    locally. No collectives (NRT CC stream has ~60-85us bootstrap).

The kernel is HBM-bound: baseline (e4m3 kq + bf16 wv) streamed 20.75MB
at the measured 357GB/s DMA roofline = 61us + 8.3us fixed startup.
This version cuts bytes to ~12.8MB and keeps PE fed continuously:

  - wv in fp8 e3m4 (4-bit mantissa): 8MB instead of 16MB bf16. e3m4
    roundoff is ~1.3% RMS on N(0,1)-scaled data (vs 2.6% for e4m3),
    which passes the 2e-2 gate with margin where e4m3 (3.2e-2) fails.
  - x-side quantization error is cancelled with a two-column stationary:
    XC = [e3m4(x) | e3m4(x - e3m4(x))] -> stationary [128, 64] gives
    pv [64, 512] = hi|lo partial products in ONE pass (PE cost is
    moving-side only), folded by one DVE add in the tail.
  - kq chain stays e4m3 and uses DoubleRow perf mode (0.5 cyc/row) so
    total PE work (~34us) fits inside the DMA window (~36us).
  - DMA spine interleaves (wkq_i, wv0_i) 0.5MB pairs so the PE's kq and
    v chains both stream without starving; wv is column-split into two
    4MB streams (cols 0:512 then 512:1024) so the first half's
    fold+matmul+store tail hides under the second half's stream.
  - bf16 tail (fold output, aT, vb preload): a@v at 1 cyc/row.

Numerics (numpy sim vs f32 reference): ~1.4e-2 rel err (gate 2e-2).
"""
import sys

for _p in (
    "/root/.axon_site",
    "/root/.axon_site/_ro/trn_rl_repo",
    "/root/.axon_site/_ro/pypackages",
):
    if _p not in sys.path:
        sys.path.insert(0, _p)

import numpy as np
from ml_dtypes import bfloat16, float8_e4m3, float8_e3m4

from concourse import bacc, tile
from concourse import mybir
from concourse.bass_utils import run_bass_kernel_spmd

W = 32          # window (seq) size
IN = 8192       # in_size
KEY = 256       # key_size
VAL = 8192      # value_size
P = 128         # SBUF partitions
NCH = IN // P   # 64 contraction chunks
NCORES = 8
VSH = VAL // NCORES   # 1024 value dims per core
KQ = 2 * KEY    # 512 = full [k | q] projection width, replicated
SCALE = 1.0 / 256.0
NT = 512        # moving free dim per matmul / column-split width
KQD = 8         # chunks per wkq DMA (0.5MB e4m3)
WVD = 8         # chunks per wv DMA (0.5MB e3m4 at NT cols)
NIT = NCH // KQD  # 8 interleave iterations

F32 = mybir.dt.float32
BF16 = mybir.dt.bfloat16
F8E4 = mybir.dt.float8e4
F8E3 = mybir.dt.float8e3
DR = mybir.MatmulPerfMode.DoubleRow

_NC = None


def _build():
    global _NC
    if _NC is not None:
        return _NC
    nc = bacc.Bacc(None, target_bir_lowering=False, debug=False, num_devices=NCORES)

    X8P = nc.declare_dram_parameter("X8P", [P, NCH // 2, 2 * P], F8E4, isOutput=False)
    XC = nc.declare_dram_parameter("XC", [P, NCH, 2 * W], F8E3, isOutput=False)
    WKQ = nc.declare_dram_parameter("WKQ", [P, NCH, KQ], F8E4, isOutput=False)
    WV0 = nc.declare_dram_parameter("WV0", [P, NCH, NT], F8E3, isOutput=False)
    WV1 = nc.declare_dram_parameter("WV1", [P, NCH, NT], F8E3, isOutput=False)
    KQB = nc.declare_dram_parameter("KQB", [1, KQ], F8E4, isOutput=False)
    VB = nc.declare_dram_parameter("VB", [1, VSH], BF16, isOutput=False)
    TRI = nc.declare_dram_parameter("TRI", [W, W], F32, isOutput=False)
    SCL = nc.declare_dram_parameter("SCL", [W, 1], F32, isOutput=False)
    SWV = nc.declare_dram_parameter("SWV", [W, 1], F32, isOutput=False)
    OUT = nc.declare_dram_parameter("out", [W, VSH], F32, isOutput=True)

    with tile.TileContext(nc) as tc:
        with (
            tc.tile_pool(name="const", bufs=1) as cpool,
            tc.tile_pool(name="kq", bufs=8) as kpool,
            tc.tile_pool(name="wv", bufs=14) as wpool,
            tc.tile_pool(name="small", bufs=1) as spool,
            tc.tile_pool(name="psum", bufs=1, space="PSUM") as ppool,
        ):
            # X8P leads the SP spine (first half, so the kq chain starts
            # ASAP); XC leads the ACT ring (needed when the first wv0 tile
            # lands ~4us later); small constants follow on the ACT ring.
            x8p_sb = cpool.tile([P, NCH // 2, 2 * P], F8E4)
            nc.sync.dma_start(out=x8p_sb[:, 0:NCH // 4, :],
                              in_=X8P[:, 0:NCH // 4, :])
            xc_sb = cpool.tile([P, NCH, 2 * W], F8E3)
            nc.scalar.dma_start(out=xc_sb[:], in_=XC[:])
            kqb_sb = cpool.tile([1, KQ], F8E4)
            nc.scalar.dma_start(out=kqb_sb[:], in_=KQB[:])
            tri_sb = cpool.tile([W, W], F32)
            nc.scalar.dma_start(out=tri_sb[:], in_=TRI[:])
            scl_sb = cpool.tile([W, 1], F32)
            nc.scalar.dma_start(out=scl_sb[:], in_=SCL[:])
            swv_sb = cpool.tile([W, 1], F32)
            nc.scalar.dma_start(out=swv_sb[:], in_=SWV[:])
            vb_sb = cpool.tile([1, VSH], BF16)
            nc.scalar.dma_start(out=vb_sb[:], in_=VB[:])
            ones1 = cpool.tile([1, W], F8E4)
            nc.vector.memset(ones1[:], 1.0)
            ones1b = cpool.tile([1, W], BF16)
            nc.vector.memset(ones1b[:], 1.0)

            pkq4 = ppool.tile([P, KQ], F32)
            pv0 = ppool.tile([2 * W, NT], F32)
            pv1 = ppool.tile([2 * W, NT], F32)
            pu0 = ppool.tile([W, NT], F32)
            pu1 = ppool.tile([W, NT], F32)

            # ---- interleaved spine, kq one tile ahead ----
            # kq chain: e4m3 DoubleRow with ZERO-PADDED M=128 stationaries.
            # DoubleRow only reaches 0.5 cyc/row with a full 128-column
            # weight load (measured: M=32 runs at exactly 1.0x); so each
            # chunk pair is packed into [128, 2, 128] with chunk c in
            # column band (c%4)*32 -> pkq4 [128, 512] holds 4 banded
            # partial kq's, folded by 3 DVE adds. v0 chain: e3m4,
            # stationary [x_hi | x_lo] -> pv0 [64, 512].
            def issue_kt(i):
                c0 = i * KQD
                kt = kpool.tile([P, KQD, KQ], F8E4, tag="kqstream")
                nc.sync.dma_start(out=kt[:], in_=WKQ[:, c0:c0 + KQD, :])
                return kt

            def kq_mms(i, kt):
                c0 = i * KQD
                for p in range(0, KQD, 2):
                    c = c0 + p
                    lt = x8p_sb[:, c // 2, :].rearrange("p (a b) -> p a b", a=2)
                    # stop on the LAST pair so the whole [128, 512] group
                    # closes (a stop covering only a partition slice leaves
                    # the rest of the bank mid-accumulation = undefined reads)
                    nc.tensor.matmul(
                        pkq4[:], lt, kt[:, p:p + 2, :],
                        start=(c == 0), stop=(c == NCH - 2), perf_mode=DR,
                    )

            def issue_wt(i):
                c0 = i * WVD
                wt = wpool.tile([P, WVD, NT], F8E3, tag="wvstream")
                nc.sync.dma_start(out=wt[:], in_=WV0[:, c0:c0 + WVD, :])
                return wt

            def v0_mms(i, wt):
                c0 = i * WVD
                for p in range(WVD):
                    c = c0 + p
                    nc.tensor.matmul(
                        pv0[:], xc_sb[:, c, :], wt[:, p, :],
                        start=(c == 0), stop=(c == NCH - 1),
                    )

            # kq chain runs CONTIGUOUSLY on the PE: interleaving normal-mode
            # matmuls between engaged-DoubleRow matmuls corrupts the DR
            # accumulation (measured), and with DR at 0.5 cyc/row the chain
            # (6.9us) hides entirely under the 4MB wkq stream (11.2us) anyway.
            kt_a = issue_kt(0)
            kt_b = issue_kt(1)
            nc.sync.dma_start(out=x8p_sb[:, NCH // 4:NCH // 2, :],
                              in_=X8P[:, NCH // 4:NCH // 2, :])
            kq_mms(0, kt_a)
            kq_mms(1, kt_b)
            kts = [issue_kt(i) for i in range(2, NIT)]
            for i, ktn in enumerate(kts):
                kq_mms(i + 2, ktn)
            # bias via 1-partition rank-1 matmul into band 0 closes the group
            nc.tensor.matmul(pkq4[0:W, :], ones1[:], kqb_sb[:],
                             start=False, stop=True, skip_group_check=True)
            # vb preload into the output PSUM groups (PE slack before v chain)
            nc.tensor.matmul(pu0[:], ones1b[:], vb_sb[:, 0:NT],
                             start=True, stop=False, skip_group_check=True)
            nc.tensor.matmul(pu1[:], ones1b[:], vb_sb[:, NT:VSH],
                             start=True, stop=False, skip_group_check=True)
            for i in range(NIT):
                wt = issue_wt(i)
                v0_mms(i, wt)

            # ---- scores + softmax (DVE/ACT work hides under the stream) ----
            # fold the 4 row bands of pkq4 into kq [32, 512]. tensor_tensor
            # needs equal base partitions, so stage each band at base 0 first.
            bands = []
            for j in range(4):
                bj = spool.tile([W, KQ], F32, tag=f"band{j}")
                nc.vector.tensor_copy(bj[:], pkq4[j * W:(j + 1) * W, :])
                bands.append(bj)
            t01 = spool.tile([W, KQ], F32)
            nc.vector.tensor_tensor(t01[:], bands[0][:], bands[1][:],
                                    mybir.AluOpType.add)
            t23 = spool.tile([W, KQ], F32)
            nc.vector.tensor_tensor(t23[:], bands[2][:], bands[3][:],
                                    mybir.AluOpType.add)
            kq_sb = spool.tile([W, KQ], F32)
            nc.vector.tensor_tensor(kq_sb[:], t01[:], t23[:],
                                    mybir.AluOpType.add)
            # 32x32 block transpose: block b holds kq[:, 32b:32b+32].T
            kqt = spool.tile([W, KQ], F32)
            nc.vector.transpose(kqt[:], kq_sb[:])
            # scores s[m,n] = sum_g q_g[:,m].T @ k_g[:,n]
            ps = ppool.tile([W, W], F32)
            for g in range(8):
                nc.tensor.matmul(
                    ps[:], kqt[:, KEY + g * W:KEY + (g + 1) * W],
                    kqt[:, g * W:(g + 1) * W],
                    start=(g == 0), stop=(g == 7),
                )
            S_sb = spool.tile([W, W], F32)
            nc.vector.tensor_copy(S_sb[:], ps[:])
            m_sb = spool.tile([W, W], F32)
            nc.vector.tensor_tensor(m_sb[:], S_sb[:], tri_sb[:], mybir.AluOpType.min)
            e_sb = spool.tile([W, W], F32)
            pz = spool.tile([W, 1], F32)
            nc.scalar.activation(
                e_sb[:], m_sb[:], mybir.ActivationFunctionType.Exp,
                scale=scl_sb[:], accum_out=pz[:],
            )
            # fold the wv dequant scale into the softmax denominator:
            # a_used = e / (Z * Swv)
            pzs = spool.tile([W, 1], F32)
            nc.vector.tensor_scalar_mul(pzs[:], pz[:], swv_sb[:])
            rz = spool.tile([W, 1], F32)
            nc.vector.reciprocal(rz[:], pzs[:])
            a_sb = spool.tile([W, W], F32)
            nc.vector.tensor_scalar_mul(a_sb[:], e_sb[:], rz[:])
            aTf = spool.tile([W, W], F32)
            nc.vector.transpose(aTf[:], a_sb[:])
            # aT stacked twice: K=64 tail matmul folds the hi|lo halves of
            # pv while applying the attention weights (one 512-row matmul)
            aT64 = spool.tile([2 * W, W], BF16)
            nc.vector.tensor_copy(aT64[0:W, :], aTf[:])
            nc.vector.tensor_copy(aT64[W:2 * W, :], aTf[:])

            # ---- wv1 stream (cols 512:1024); tail0 hides under it ----
            did_tail0 = False
            for i in range(NIT - 1):
                c0 = i * WVD
                wt = wpool.tile([P, WVD, NT], F8E3, tag="wvstream")
                nc.sync.dma_start(out=wt[:], in_=WV1[:, c0:c0 + WVD, :])
                for p in range(WVD):
                    c = c0 + p
                    nc.tensor.matmul(
                        pv1[:], xc_sb[:, c, :], wt[:, p, :],
                        start=(c == 0), stop=False,
                    )
                if i == 1 and not did_tail0:
                    # tail0: hi|lo fold rides the K=64 a@v matmul onto the
                    # vb preload, store on the ACT ring (hidden under wv1)
                    did_tail0 = True
                    v0_sb = spool.tile([2 * W, NT], BF16, tag="v0")
                    nc.vector.tensor_copy(v0_sb[:], pv0[:])
                    nc.tensor.matmul(pu0[:], aT64[:], v0_sb[:],
                                     start=False, stop=True,
                                     skip_group_check=True)
                    o0_sb = spool.tile([W, NT], F32, tag="o0")
                    nc.vector.tensor_copy(o0_sb[:], pu0[:])
                    nc.scalar.dma_start(out=OUT[:, 0:NT], in_=o0_sb[:])
            # last tile split 4/2/1/1 so the final bytes land earlier
            c = NCH - WVD
            for sz in (4, 2, 1, 1):
                wt1 = wpool.tile([P, sz, NT], F8E3, tag="wvstream")
                nc.sync.dma_start(out=wt1[:], in_=WV1[:, c:c + sz, :])
                for p in range(sz):
                    nc.tensor.matmul(
                        pv1[:], xc_sb[:, c + p, :], wt1[:, p, :],
                        start=False, stop=(c + p == NCH - 1),
                    )
                c += sz

            # ---- tail1: two 256-col pieces pipelined across DVE/PE/DMA ----
            HT = NT // 2
            v1_sb = spool.tile([2 * W, NT], BF16, tag="v1")
            o1_sb = spool.tile([W, NT], F32, tag="o1")
            nc.vector.tensor_copy(v1_sb[:, 0:HT], pv1[:, 0:HT])
            nc.tensor.matmul(pu1[:, 0:HT], aT64[:], v1_sb[:, 0:HT],
                             start=False, stop=True, skip_group_check=True)
            nc.vector.tensor_copy(v1_sb[:, HT:NT], pv1[:, HT:NT])
            nc.vector.tensor_copy(o1_sb[:, 0:HT], pu1[:, 0:HT])
            nc.sync.dma_start(out=OUT[:, NT:NT + HT], in_=o1_sb[:, 0:HT])
            nc.tensor.matmul(pu1[:, HT:NT], aT64[:], v1_sb[:, HT:NT],
                             start=False, stop=True, skip_group_check=True)
            nc.vector.tensor_copy(o1_sb[:, HT:NT], pu1[:, HT:NT])
            nc.sync.dma_start(out=OUT[:, NT + HT:VSH], in_=o1_sb[:, HT:NT])

    nc.compile()
    _NC = nc
    return nc


def _swz(mat_t, dt):
    """[rows=IN, cols] (IN-major) -> [P, NCH, cols] in dtype dt."""
    rows, cols = mat_t.shape
    assert rows == IN
    return np.ascontiguousarray(
        mat_t.reshape(NCH, P, cols).transpose(1, 0, 2).astype(dt))


def _e4(a):
    return np.clip(a, -240.0, 240.0).astype(float8_e4m3)


def _e3(a):
    return np.clip(a, -15.5, 15.5).astype(float8_e3m4)


def _make_in_maps(x, wk_w, wk_b, wq_w, wq_b, wv_w, wv_b, tri):
    x = np.asarray(x, dtype=np.float32)
    xT = np.ascontiguousarray(x.T)
    # Zero-padded M=128 DoubleRow stationaries: pair t covers chunks
    # (2t, 2t+1); chunk c sits in column band (c%4)*32 of its pair
    # element, so pkq4 row band j accumulates sum over chunks c==j (mod 4).
    x8c = _e4(xT).astype(np.float32).reshape(NCH, P, W)
    X8P_f = np.zeros((NCH // 2, P, 2, P), dtype=np.float32)
    X8P_f[0::2, :, 0, 0:W] = x8c[0::4]
    X8P_f[0::2, :, 1, W:2 * W] = x8c[1::4]
    X8P_f[1::2, :, 0, 2 * W:3 * W] = x8c[2::4]
    X8P_f[1::2, :, 1, 3 * W:4 * W] = x8c[3::4]
    X8P_dev = np.ascontiguousarray(
        X8P_f.reshape(NCH // 2, P, 2 * P).transpose(1, 0, 2)
        .astype(float8_e4m3))
    # e3m4 hi/lo split of x: XC = [hi | lo] per chunk -> stationary [128, 64]
    x_hi = _e3(xT).astype(np.float32)
    x_lo = _e3(xT - x_hi).astype(np.float32)
    XC_f = np.concatenate(
        [x_hi.reshape(NCH, P, W), x_lo.reshape(NCH, P, W)], axis=2)
    XC_dev = np.ascontiguousarray(
        XC_f.transpose(1, 0, 2).astype(float8_e3m4))
    # fp8 e4m3 kq path: prescale wk/wq (and biases) by S so values sit
    # ~N(0,1); scores come out S^2-scaled and 1/(256*S^2) is applied inside
    # the exp (runtime scale tensor), with tri pre-scaled to match.
    wkq = np.concatenate([np.asarray(wk_w, dtype=np.float32),
                          np.asarray(wq_w, dtype=np.float32)], axis=0)
    S = 1.0 / max(float(np.std(wkq)), 1e-12)
    TRI = np.ascontiguousarray(
        np.asarray(tri, dtype=np.float32) * (256.0 * S * S))
    SCL = np.full((W, 1), SCALE / (S * S), dtype=np.float32)
    WKQ_dev = _swz(np.clip(np.ascontiguousarray(wkq.T) * S, -240.0, 240.0)
                   .astype(float8_e4m3).astype(np.float32), float8_e4m3)
    KQB_dev = np.ascontiguousarray(_e4(np.concatenate([
        np.asarray(wk_b, dtype=np.float32),
        np.asarray(wq_b, dtype=np.float32),
    ]) * S).reshape(1, KQ))
    # e3m4 wv path: global scale Swv so values sit ~N(0,1); the dequant
    # 1/Swv is folded into the softmax denominator on device (SWV tensor).
    wv_w = np.asarray(wv_w, dtype=np.float32)
    Swv = 1.0 / max(float(np.std(wv_w)), 1e-12)
    SWVt = np.full((W, 1), Swv, dtype=np.float32)
    in_maps = []
    for c in range(NCORES):
        wv_sh = np.ascontiguousarray(wv_w[c * VSH:(c + 1) * VSH, :].T) * Swv
        wv_q = np.clip(wv_sh, -15.5, 15.5).astype(float8_e3m4)
        in_maps.append({
            "X8P": X8P_dev,
            "XC": XC_dev,
            "WKQ": WKQ_dev,
            "WV0": np.ascontiguousarray(
                wv_q[:, 0:NT].reshape(NCH, P, NT).transpose(1, 0, 2)),
            "WV1": np.ascontiguousarray(
                wv_q[:, NT:VSH].reshape(NCH, P, NT).transpose(1, 0, 2)),
            "KQB": KQB_dev,
            "VB": np.ascontiguousarray(
                np.asarray(wv_b[c * VSH:(c + 1) * VSH], dtype=np.float32)
                .reshape(1, VSH).astype(bfloat16)),
            "TRI": TRI,
            "SCL": SCL,
            "SWV": SWVt,
        })
    return in_maps


def run(inputs, trace=False):
    """Build + run on 8 cores; returns (full_output, BassKernelResults)."""
    nc = _build()
    in_maps = _make_in_maps(**inputs)
    res = run_bass_kernel_spmd(
        nc, in_maps, core_ids=list(range(NCORES)), trace=trace,
    )
    out = np.concatenate([res.results[c]["out"] for c in range(NCORES)], axis=1)
    return out, res


def kernel(**inputs):
    out, _ = run(inputs, trace=False)
    return out


if __name__ == "__main__":
    rng = np.random.default_rng(0)
    ins = {
        "x": rng.standard_normal((W, IN), dtype=np.float32),
        "wk_w": rng.standard_normal((KEY, IN), dtype=np.float32) / 90.5,
        "wk_b": rng.standard_normal((KEY,), dtype=np.float32) / 90.5,
        "wq_w": rng.standard_normal((KEY, IN), dtype=np.float32) / 90.5,
        "wq_b": rng.standard_normal((KEY,), dtype=np.float32) / 90.5,
        "wv_w": rng.standard_normal((VAL, IN), dtype=np.float32) / 90.5,
        "wv_b": rng.standard_normal((VAL,), dtype=np.float32) / 90.5,
        "tri": ((np.tril(np.full((W, W), 2.0, dtype=np.float32)) - 1.0) * 1e5),
    }
    out = kernel(**ins)
    print("out", out.shape, out.dtype, np.abs(out).mean())


# revision 33
# speedup vs baseline: 1.0210x; 1.0210x over previous
"""Trainium2 Bass kernel for nn_AttentionHead_17042430231165.

out = softmax(min((x@wq.T+qb)@(x@wk.T+kb).T / 256, tri)) @ (x@wv.T+vb)
  x [32, 8192], wk/wq [256, 8192], wv [8192, 8192], tri [32, 32]

Sharding (8 cores):
  - wv rows (VAL) sharded: 1024 rows/core -> each core computes out[:, c*1024:(c+1)*1024]
  - wk/wq fully REPLICATED on every core; scores and softmax computed
    locally. No collectives (NRT CC stream has ~60-85us bootstrap).

The kernel is HBM-bound: baseline (e4m3 kq + bf16 wv) streamed 20.75MB
at the measured 357GB/s DMA roofline = 61us + 8.3us fixed startup.
This version cuts bytes to ~12.8MB and keeps PE fed continuously:

  - wv in fp8 e3m4 (4-bit mantissa): 8MB instead of 16MB bf16. e3m4
    roundoff is ~1.3% RMS on N(0,1)-scaled data (vs 2.6% for e4m3),
    which passes the 2e-2 gate with margin where e4m3 (3.2e-2) fails.
  - x-side quantization error is cancelled with a two-column stationary:
    XC = [e3m4(x) | e3m4(x - e3m4(x))] -> stationary [128, 64] gives
    pv [64, 512] = hi|lo partial products in ONE pass (PE cost is
    moving-side only), folded by one DVE add in the tail.
  - kq chain stays e4m3 and uses DoubleRow perf mode (0.5 cyc/row) so
    total PE work (~34us) fits inside the DMA window (~36us).
  - DMA spine interleaves (wkq_i, wv0_i) 0.5MB pairs so the PE's kq and
    v chains both stream without starving; wv is column-split into two
    4MB streams (cols 0:512 then 512:1024) so the first half's
    fold+matmul+store tail hides under the second half's stream.
  - bf16 tail (fold output, aT, vb preload): a@v at 1 cyc/row.

Numerics (numpy sim vs f32 reference): ~1.4e-2 rel err (gate 2e-2).
"""
import sys

for _p in (
    "/root/.axon_site",
    "/root/.axon_site/_ro/trn_rl_repo",
    "/root/.axon_site/_ro/pypackages",
):
    if _p not in sys.path:
        sys.path.insert(0, _p)

import numpy as np
from ml_dtypes import bfloat16, float8_e4m3, float8_e3m4

from concourse import bacc, tile
from concourse import mybir
from concourse.bass_utils import run_bass_kernel_spmd

W = 32          # window (seq) size
IN = 8192       # in_size
KEY = 256       # key_size
VAL = 8192      # value_size
P = 128         # SBUF partitions
NCH = IN // P   # 64 contraction chunks
NCORES = 8
VSH = VAL // NCORES   # 1024 value dims per core
KQ = 2 * KEY    # 512 = full [k | q] projection width, replicated
SCALE = 1.0 / 256.0
NT = 512        # moving free dim per matmul / column-split width
KQD = 8         # chunks per wkq DMA (0.5MB e4m3)
WVD = 8         # chunks per wv DMA (0.5MB e3m4 at NT cols)
NIT = NCH // KQD  # 8 interleave iterations

F32 = mybir.dt.float32
BF16 = mybir.dt.bfloat16
F8E4 = mybir.dt.float8e4
F8E3 = mybir.dt.float8e3
DR = mybir.MatmulPerfMode.DoubleRow

_NC = None


def _build():
    global _NC
    if _NC is not None:
        return _NC
    nc = bacc.Bacc(None, target_bir_lowering=False, debug=False, num_devices=NCORES)

    X8P = nc.declare_dram_parameter("X8P", [P, NCH // 2, 2 * P], F8E4, isOutput=False)
    XC = nc.declare_dram_parameter("XC", [P, NCH, 2 * W], F8E3, isOutput=False)
    WKQ = nc.declare_dram_parameter("WKQ", [P, NCH, KQ], F8E4, isOutput=False)
    WV0 = nc.declare_dram_parameter("WV0", [P, NCH, NT], F8E3, isOutput=False)
    WV1 = nc.declare_dram_parameter("WV1", [P, NCH, NT], F8E3, isOutput=False)
    KQB = nc.declare_dram_parameter("KQB", [1, KQ], F8E4, isOutput=False)
    VB = nc.declare_dram_parameter("VB", [1, VSH], BF16, isOutput=False)
    TRI = nc.declare_dram_parameter("TRI", [W, W], F32, isOutput=False)
    SCL = nc.declare_dram_parameter("SCL", [W, 1], F32, isOutput=False)
    SWV = nc.declare_dram_parameter("SWV", [W, 1], F32, isOutput=False)
    OUT = nc.declare_dram_parameter("out", [W, VSH], F32, isOutput=True)

    with tile.TileContext(nc) as tc:
        with (
            tc.tile_pool(name="const", bufs=1) as cpool,
            tc.tile_pool(name="kq", bufs=8) as kpool,
            tc.tile_pool(name="wv", bufs=14) as wpool,
            tc.tile_pool(name="small", bufs=1) as spool,
            tc.tile_pool(name="psum", bufs=1, space="PSUM") as ppool,
        ):
            # X8P leads the SP spine (first half, so the kq chain starts
            # ASAP); XC leads the ACT ring (needed when the first wv0 tile
            # lands ~4us later); small constants follow on the ACT ring.
            x8p_sb = cpool.tile([P, NCH // 2, 2 * P], F8E4)
            nc.sync.dma_start(out=x8p_sb[:, 0:NCH // 8, :],
                              in_=X8P[:, 0:NCH // 8, :])
            xc_sb = cpool.tile([P, NCH, 2 * W], F8E3)
            nc.scalar.dma_start(out=xc_sb[:], in_=XC[:])
            kqb_sb = cpool.tile([1, KQ], F8E4)
            nc.scalar.dma_start(out=kqb_sb[:], in_=KQB[:])
            tri_sb = cpool.tile([W, W], F32)
            nc.scalar.dma_start(out=tri_sb[:], in_=TRI[:])
            scl_sb = cpool.tile([W, 1], F32)
            nc.scalar.dma_start(out=scl_sb[:], in_=SCL[:])
            swv_sb = cpool.tile([W, 1], F32)
            nc.scalar.dma_start(out=swv_sb[:], in_=SWV[:])
            vb_sb = cpool.tile([1, VSH], BF16)
            nc.scalar.dma_start(out=vb_sb[:], in_=VB[:])
            ones1 = cpool.tile([1, W], F8E4)
            nc.vector.memset(ones1[:], 1.0)
            ones1b = cpool.tile([1, W], BF16)
            nc.vector.memset(ones1b[:], 1.0)

            pkq4 = ppool.tile([P, KQ], F32)
            pv0 = ppool.tile([2 * W, NT], F32)
            pv1 = ppool.tile([2 * W, NT], F32)
            pu0 = ppool.tile([W, NT], F32)
            pu1 = ppool.tile([W, NT], F32)

            # ---- interleaved spine, kq one tile ahead ----
            # kq chain: e4m3 DoubleRow with ZERO-PADDED M=128 stationaries.
            # DoubleRow only reaches 0.5 cyc/row with a full 128-column
            # weight load (measured: M=32 runs at exactly 1.0x); so each
            # chunk pair is packed into [128, 2, 128] with chunk c in
            # column band (c%4)*32 -> pkq4 [128, 512] holds 4 banded
            # partial kq's, folded by 3 DVE adds. v0 chain: e3m4,
            # stationary [x_hi | x_lo] -> pv0 [64, 512].
            def issue_kt(i):
                c0 = i * KQD
                kt = kpool.tile([P, KQD, KQ], F8E4, tag="kqstream")
                nc.sync.dma_start(out=kt[:], in_=WKQ[:, c0:c0 + KQD, :])
                return kt

            def kq_mms(i, kt):
                c0 = i * KQD
                for p in range(0, KQD, 2):
                    c = c0 + p
                    lt = x8p_sb[:, c // 2, :].rearrange("p (a b) -> p a b", a=2)
                    # stop on the LAST pair so the whole [128, 512] group
                    # closes (a stop covering only a partition slice leaves
                    # the rest of the bank mid-accumulation = undefined reads)
                    nc.tensor.matmul(
                        pkq4[:], lt, kt[:, p:p + 2, :],
                        start=(c == 0), stop=(c == NCH - 2), perf_mode=DR,
                    )

            def issue_wt(i):
                c0 = i * WVD
                wt = wpool.tile([P, WVD, NT], F8E3, tag="wvstream")
                nc.sync.dma_start(out=wt[:], in_=WV0[:, c0:c0 + WVD, :])
                return wt

            def v0_mms(i, wt):
                c0 = i * WVD
                for p in range(WVD):
                    c = c0 + p
                    nc.tensor.matmul(
                        pv0[:], xc_sb[:, c, :], wt[:, p, :],
                        start=(c == 0), stop=(c == NCH - 1),
                    )

            # Interleaved spine with the kq chain one tile ahead: PE fills
            # the gaps between kt arrivals with v0 work and consumes wv0
            # tiles as they land, so it never starves.
            kt_a = issue_kt(0)
            kt_b = issue_kt(1)
            nc.sync.dma_start(out=x8p_sb[:, NCH // 8:NCH // 2, :],
                              in_=X8P[:, NCH // 8:NCH // 2, :])
            kq_mms(0, kt_a)
            kq_mms(1, kt_b)
            for i in range(NIT - 2):
                wt = issue_wt(i)
                ktn = issue_kt(i + 2)
                v0_mms(i, wt)
                kq_mms(i + 2, ktn)
            # bias via 1-partition rank-1 matmul into band 0 closes the group
            nc.tensor.matmul(pkq4[0:W, :], ones1[:], kqb_sb[:],
                             start=False, stop=True, skip_group_check=True)
            # vb preload into the output PSUM groups (PE slack)
            nc.tensor.matmul(pu0[:], ones1b[:], vb_sb[:, 0:NT],
                             start=True, stop=False, skip_group_check=True)
            nc.tensor.matmul(pu1[:], ones1b[:], vb_sb[:, NT:VSH],
                             start=True, stop=False, skip_group_check=True)
            for i in (NIT - 2, NIT - 1):
                wt = issue_wt(i)
                v0_mms(i, wt)

            # ---- scores + softmax (DVE/ACT work hides under the stream) ----
            # fold the 4 row bands of pkq4 into kq [32, 512]. tensor_tensor
            # needs equal base partitions, so stage each band at base 0 first.
            bands = []
            for j in range(4):
                bj = spool.tile([W, KQ], F32, tag=f"band{j}")
                nc.vector.tensor_copy(bj[:], pkq4[j * W:(j + 1) * W, :])
                bands.append(bj)
            t01 = spool.tile([W, KQ], F32)
            nc.vector.tensor_tensor(t01[:], bands[0][:], bands[1][:],
                                    mybir.AluOpType.add)
            t23 = spool.tile([W, KQ], F32)
            nc.vector.tensor_tensor(t23[:], bands[2][:], bands[3][:],
                                    mybir.AluOpType.add)
            kq_sb = spool.tile([W, KQ], F32)
            nc.vector.tensor_tensor(kq_sb[:], t01[:], t23[:],
                                    mybir.AluOpType.add)
            # 32x32 block transpose: block b holds kq[:, 32b:32b+32].T
            kqt = spool.tile([W, KQ], F32)
            nc.vector.transpose(kqt[:], kq_sb[:])
            # scores s[m,n] = sum_g q_g[:,m].T @ k_g[:,n]
            ps = ppool.tile([W, W], F32)
            for g in range(8):
                nc.tensor.matmul(
                    ps[:], kqt[:, KEY + g * W:KEY + (g + 1) * W],
                    kqt[:, g * W:(g + 1) * W],
                    start=(g == 0), stop=(g == 7),
                )
            S_sb = spool.tile([W, W], F32)
            nc.vector.tensor_copy(S_sb[:], ps[:])
            m_sb = spool.tile([W, W], F32)
            nc.vector.tensor_tensor(m_sb[:], S_sb[:], tri_sb[:], mybir.AluOpType.min)
            e_sb = spool.tile([W, W], F32)
            pz = spool.tile([W, 1], F32)
            nc.scalar.activation(
                e_sb[:], m_sb[:], mybir.ActivationFunctionType.Exp,
                scale=scl_sb[:], accum_out=pz[:],
            )
            # fold the wv dequant scale into the softmax denominator:
            # a_used = e / (Z * Swv)
            pzs = spool.tile([W, 1], F32)
            nc.vector.tensor_scalar_mul(pzs[:], pz[:], swv_sb[:])
            rz = spool.tile([W, 1], F32)
            nc.vector.reciprocal(rz[:], pzs[:])
            a_sb = spool.tile([W, W], F32)
            nc.vector.tensor_scalar_mul(a_sb[:], e_sb[:], rz[:])
            aTf = spool.tile([W, W], F32)
            nc.vector.transpose(aTf[:], a_sb[:])
            # aT stacked twice: K=64 tail matmul folds the hi|lo halves of
            # pv while applying the attention weights (one 512-row matmul)
            aT64 = spool.tile([2 * W, W], BF16)
            nc.vector.tensor_copy(aT64[0:W, :], aTf[:])
            nc.vector.tensor_copy(aT64[W:2 * W, :], aTf[:])

            # ---- wv1 stream (cols 512:1024); tail0 hides under it ----
            did_tail0 = False
            for i in range(NIT - 1):
                c0 = i * WVD
                wt = wpool.tile([P, WVD, NT], F8E3, tag="wvstream")
                nc.sync.dma_start(out=wt[:], in_=WV1[:, c0:c0 + WVD, :])
                for p in range(WVD):
                    c = c0 + p
                    nc.tensor.matmul(
                        pv1[:], xc_sb[:, c, :], wt[:, p, :],
                        start=(c == 0), stop=False,
                    )
                if i == 3 and not did_tail0:
                    # tail0: hi|lo fold rides the K=64 a@v matmul onto the
                    # vb preload, store on the ACT ring (hidden under wv1)
                    did_tail0 = True
                    v0_sb = spool.tile([2 * W, NT], BF16, tag="v0")
                    nc.vector.tensor_copy(v0_sb[:], pv0[:])
                    nc.tensor.matmul(pu0[:], aT64[:], v0_sb[:],
                                     start=False, stop=True,
                                     skip_group_check=True)
                    o0_sb = spool.tile([W, NT], F32, tag="o0")
                    nc.vector.tensor_copy(o0_sb[:], pu0[:])
                    nc.scalar.dma_start(out=OUT[:, 0:NT], in_=o0_sb[:])
            # last tile split 4/2/1/1 so the final bytes land earlier
            c = NCH - WVD
            for sz in (4, 2, 1, 1):
                wt1 = wpool.tile([P, sz, NT], F8E3, tag="wvstream")
                nc.sync.dma_start(out=wt1[:], in_=WV1[:, c:c + sz, :])
                for p in range(sz):
                    nc.tensor.matmul(
                        pv1[:], xc_sb[:, c + p, :], wt1[:, p, :],
                        start=False, stop=(c + p == NCH - 1),
                    )
                c += sz

            # ---- tail1: two 256-col pieces pipelined across DVE/PE/DMA ----
            HT = NT // 2
            v1_sb = spool.tile([2 * W, NT], BF16, tag="v1")
            o1_sb = spool.tile([W, NT], F32, tag="o1")
            nc.vector.tensor_copy(v1_sb[:, 0:HT], pv1[:, 0:HT])
            nc.tensor.matmul(pu1[:, 0:HT], aT64[:], v1_sb[:, 0:HT],
                             start=False, stop=True, skip_group_check=True)
            nc.vector.tensor_copy(v1_sb[:, HT:NT], pv1[:, HT:NT])
            nc.vector.tensor_copy(o1_sb[:, 0:HT], pu1[:, 0:HT])
            nc.sync.dma_start(out=OUT[:, NT:NT + HT], in_=o1_sb[:, 0:HT])
            nc.tensor.matmul(pu1[:, HT:NT], aT64[:], v1_sb[:, HT:NT],
                             start=False, stop=True, skip_group_check=True)
            nc.vector.tensor_copy(o1_sb[:, HT:NT], pu1[:, HT:NT])
            nc.sync.dma_start(out=OUT[:, NT + HT:VSH], in_=o1_sb[:, HT:NT])

    nc.compile()
    _NC = nc
    return nc


def _swz(mat_t, dt):
    """[rows=IN, cols] (IN-major) -> [P, NCH, cols] in dtype dt."""
    rows, cols = mat_t.shape
    assert rows == IN
    return np.ascontiguousarray(
        mat_t.reshape(NCH, P, cols).transpose(1, 0, 2).astype(dt))


def _e4(a):
    return np.clip(a, -240.0, 240.0).astype(float8_e4m3)


def _e3(a):
    return np.clip(a, -15.5, 15.5).astype(float8_e3m4)


def _make_in_maps(x, wk_w, wk_b, wq_w, wq_b, wv_w, wv_b, tri):
    x = np.asarray(x, dtype=np.float32)
    xT = np.ascontiguousarray(x.T)
    # Zero-padded M=128 DoubleRow stationaries: pair t covers chunks
    # (2t, 2t+1); chunk c sits in column band (c%4)*32 of its pair
    # element, so pkq4 row band j accumulates sum over chunks c==j (mod 4).
    x8c = _e4(xT).astype(np.float32).reshape(NCH, P, W)
    X8P_f = np.zeros((NCH // 2, P, 2, P), dtype=np.float32)
    X8P_f[0::2, :, 0, 0:W] = x8c[0::4]
    X8P_f[0::2, :, 1, W:2 * W] = x8c[1::4]
    X8P_f[1::2, :, 0, 2 * W:3 * W] = x8c[2::4]
    X8P_f[1::2, :, 1, 3 * W:4 * W] = x8c[3::4]
    X8P_dev = np.ascontiguousarray(
        X8P_f.reshape(NCH // 2, P, 2 * P).transpose(1, 0, 2)
        .astype(float8_e4m3))
    # e3m4 hi/lo split of x: XC = [hi | lo] per chunk -> stationary [128, 64]
    x_hi = _e3(xT).astype(np.float32)
    x_lo = _e3(xT - x_hi).astype(np.float32)
    XC_f = np.concatenate(
        [x_hi.reshape(NCH, P, W), x_lo.reshape(NCH, P, W)], axis=2)
    XC_dev = np.ascontiguousarray(
        XC_f.transpose(1, 0, 2).astype(float8_e3m4))
    # fp8 e4m3 kq path: prescale wk/wq (and biases) by S so values sit
    # ~N(0,1); scores come out S^2-scaled and 1/(256*S^2) is applied inside
    # the exp (runtime scale tensor), with tri pre-scaled to match.
    wkq = np.concatenate([np.asarray(wk_w, dtype=np.float32),
                          np.asarray(wq_w, dtype=np.float32)], axis=0)
    S = 1.0 / max(float(np.std(wkq)), 1e-12)
    TRI = np.ascontiguousarray(
        np.asarray(tri, dtype=np.float32) * (256.0 * S * S))
    SCL = np.full((W, 1), SCALE / (S * S), dtype=np.float32)
    WKQ_dev = _swz(np.clip(np.ascontiguousarray(wkq.T) * S, -240.0, 240.0)
                   .astype(float8_e4m3).astype(np.float32), float8_e4m3)
    KQB_dev = np.ascontiguousarray(_e4(np.concatenate([
        np.asarray(wk_b, dtype=np.float32),
        np.asarray(wq_b, dtype=np.float32),
    ]) * S).reshape(1, KQ))
    # e3m4 wv path: global scale Swv so values sit ~N(0,1); the dequant
    # 1/Swv is folded into the softmax denominator on device (SWV tensor).
    wv_w = np.asarray(wv_w, dtype=np.float32)
    Swv = 1.0 / max(float(np.std(wv_w)), 1e-12)
    SWVt = np.full((W, 1), Swv, dtype=np.float32)
    in_maps = []
    for c in range(NCORES):
        wv_sh = np.ascontiguousarray(wv_w[c * VSH:(c + 1) * VSH, :].T) * Swv
        wv_q = np.clip(wv_sh, -15.5, 15.5).astype(float8_e3m4)
        in_maps.append({
            "X8P": X8P_dev,
            "XC": XC_dev,
            "WKQ": WKQ_dev,
            "WV0": np.ascontiguousarray(
                wv_q[:, 0:NT].reshape(NCH, P, NT).transpose(1, 0, 2)),
            "WV1": np.ascontiguousarray(
                wv_q[:, NT:VSH].reshape(NCH, P, NT).transpose(1, 0, 2)),
            "KQB": KQB_dev,
            "VB": np.ascontiguousarray(
                np.asarray(wv_b[c * VSH:(c + 1) * VSH], dtype=np.float32)
                .reshape(1, VSH).astype(bfloat16)),
            "TRI": TRI,
            "SCL": SCL,
            "SWV": SWVt,
        })
    return in_maps


def run(inputs, trace=False):
    """Build + run on 8 cores; returns (full_output, BassKernelResults)."""
    nc = _build()
    in_maps = _make_in_maps(**inputs)
    res = run_bass_kernel_spmd(
        nc, in_maps, core_ids=list(range(NCORES)), trace=trace,
    )
    out = np.concatenate([res.results[c]["out"] for c in range(NCORES)], axis=1)
    return out, res


def kernel(**inputs):
    out, _ = run(inputs, trace=False)
    return out


if __name__ == "__main__":
    rng = np.random.default_rng(0)
    ins = {
        "x": rng.standard_normal((W, IN), dtype=np.float32),
        "wk_w": rng.standard_normal((KEY, IN), dtype=np.float32) / 90.5,
        "wk_b": rng.standard_normal((KEY,), dtype=np.float32) / 90.5,
        "wq_w": rng.standard_normal((KEY, IN), dtype=np.float32) / 90.5,
        "wq_b": rng.standard_normal((KEY,), dtype=np.float32) / 90.5,
        "wv_w": rng.standard_normal((VAL, IN), dtype=np.float32) / 90.5,
        "wv_b": rng.standard_normal((VAL,), dtype=np.float32) / 90.5,
        "tri": ((np.tril(np.full((W, W), 2.0, dtype=np.float32)) - 1.0) * 1e5),
    }
    out = kernel(**ins)
    print("out", out.shape, out.dtype, np.abs(out).mean())
